# revision 4
# baseline (speedup 1.0000x reference)
"""Trainium2 Bass kernel for 2-layer TransformerConv GNN + mean-pool head.

Strategy (SPMD over 8 cores):
  * Host sorts edges by dst, shards by dst-range (edge-balanced, snapped to
    node boundaries). Each core owns a contiguous dst node range; its Z
    (softmax numerator/denominator sums per node) is complete locally.
  * Edge tiles of 128 edges, dst-ALIGNED (no dst straddles a tile), so the
    per-tile selection-matrix matmul produces COMPLETE per-dst sums and a
    bypass indirect scatter (duplicate rows carry identical values) needs
    no cross-tile accumulation.
  * logits = q[dst]·k[src] + ea*(q[dst]·We)  (qWe precomputed per node);
    softmax without max-subtraction (logits are O(1); exp is safe);
    alpha folding: Z = [Σ ex*v | Σ ex | Σ ex*ea] per node, divided at node
    level, so no per-edge denominator gather.
  * Layer-1 node tables: each core computes its shard from its Z, then one
    AllGather shares the (k|v) table; q/skip tables stay local (dst-local).
  * Final: per-core partial graph sums via accumulated one-hot matmuls,
    one AllReduce [64,128], replicated tiny MLP head.
"""
import sys
sys.path.insert(0, '/opt/trn_rl_repo')
import copy as _copy
import numpy as np

import concourse.bass as bass
import concourse.tile as tile
from concourse import mybir
from concourse.tile_rust import add_dep_helper
from concourse.bass_utils import run_bass_kernel_spmd
import bass_rust as _bass_rust

# ---------------- problem constants (hardcoded per spec) ----------------
H, D, HD = 2, 64, 128
N_NODES, N_EDGES, N_GRAPHS = 50000, 800000, 64
IN_CH, EDGE_DIM, N_CLASSES = 2, 1, 30
NC_CORES = 8
P = 128
B = 4           # edge tiles per batch
CH = 32         # edge tiles per index-chunk load
F32 = mybir.dt.float32
I32 = mybir.dt.int32

NPAD = ((N_NODES + 127) // 128) * 128          # 50048


# ---------------- walrus sync-wait legalization (this toolchain caps
# embedded waits at 1 per instruction, 2 for EventSemaphore) --------------
def _legalize_waits(nc):
    m = nc.m
    new_module = _copy.replace(m, functions=[])
    for function in m.functions:
        nf = _copy.replace(function, blocks=[])
        nf.set_allocations_from_list(function.allocations)
        for block in function.blocks:
            insts = []
            for inst in block.instructions:
                si = inst.sync_info
                waits = list(si.on_wait) if si and si.on_wait else []
                cap = 2 if isinstance(inst, mybir.InstEventSemaphore) else 1
                if len(waits) > cap:
                    extra, keep = waits[:-cap], waits[-cap:]
                    for i in range(0, len(extra), 2):
                        insts.append(mybir.InstEventSemaphore(
                            name=f"{inst.name}-wait{i}",
                            engine=inst.engine,
                            sync_info=_bass_rust.SyncInfo(
                                on_wait=extra[i:i + 2], on_update=[]),
                        ))
                    inst.sync_info = _bass_rust.SyncInfo(
                        on_wait=keep,
                        on_update=list(si.on_update) if si.on_update else [])
                insts.append(inst)
            nf.blocks.append(_copy.replace(block, instructions=insts))
        new_module.functions.append(nf)
    nc.m = new_module


# ---------------- host-side sharding ----------------
def _host_prep(inputs):
    src = np.asarray(inputs['edge_index'][0]).astype(np.int64)
    dst = np.asarray(inputs['edge_index'][1]).astype(np.int64)
    ea = np.asarray(inputs['edge_attr'][:, 0]).astype(np.float32)
    order = np.argsort(dst, kind='stable')
    src, dst, ea = src[order], dst[order], ea[order]

    # core boundaries: ~E/8 edges each, snapped so no dst node straddles
    cuts = [0]
    for c in range(1, NC_CORES):
        e = c * N_EDGES // NC_CORES
        while e < N_EDGES and e > 0 and dst[e] == dst[e - 1]:
            e += 1
        cuts.append(e)
    cuts.append(N_EDGES)

    per_core = []
    for c in range(NC_CORES):
        lo, hi = cuts[c], cuts[c + 1]
        s, d, a = src[lo:hi], dst[lo:hi], ea[lo:hi]
        n_lo = int(d[0]) if hi > lo else 0
        n_hi = int(d[-1]) + 1 if hi > lo else 1
        # dst-aligned tiles of <=128 edges (greedy pack whole-dst groups)
        bnd = np.flatnonzero(np.diff(d)) + 1
        groups = np.split(np.arange(hi - lo), bnd)
        tiles, cur, cnt = [], [], 0
        for g in groups:
            if cnt + len(g) > P:
                tiles.append(np.concatenate(cur))
                cur, cnt = [], 0
            assert len(g) <= P, "node degree > 128 unsupported"
            cur.append(g)
            cnt += len(g)
        if cur:
            tiles.append(np.concatenate(cur))
        per_core.append(dict(s=s, d=d, a=a, n_lo=n_lo, n_hi=n_hi, tiles=tiles))

    T = max(len(pc['tiles']) for pc in per_core)
    T = ((T + CH - 1) // CH) * CH
    NLOC = max(pc['n_hi'] - pc['n_lo'] for pc in per_core)
    NLOC = ((NLOC + 2 + 127) // 128) * 128          # +dummy row, 128-mult
    DUMMY = NLOC - 1

    cores = []
    for pc in per_core:
        srcg = np.zeros((T, P), np.int32)
        dstg = np.zeros((T, P), np.int32)
        dstl = np.full((T, P), DUMMY, np.int32)
        eav = np.zeros((T, P), np.float32)
        for t, idx in enumerate(pc['tiles']):
            k = len(idx)
            srcg[t, :k] = pc['s'][idx]
            dstg[t, :k] = pc['d'][idx]
            dstl[t, :k] = pc['d'][idx] - pc['n_lo']
        for t, idx in enumerate(pc['tiles']):
            eav[t, :len(idx)] = pc['a'][idx]
        cores.append(dict(srcg=srcg, dstg=dstg, dstl=dstl, eav=eav,
                          n_lo=pc['n_lo'], n_hi=pc['n_hi']))

    # AllGather-space src index: core c's rows live at c*NLOC + (n - n_lo[c])
    node_core = np.zeros(N_NODES, np.int64)
    node_loc = np.zeros(N_NODES, np.int64)
    for c, pc in enumerate(cores):
        node_core[pc['n_lo']:pc['n_hi']] = c
        node_loc[pc['n_lo']:pc['n_hi']] = (
            np.arange(pc['n_lo'], pc['n_hi']) - pc['n_lo'] + c * NLOC)
    for c, pc in enumerate(cores):
        sg = pc['srcg']
        pc['srcag'] = node_loc[sg].astype(np.int32)

    # per-core own-node global ids (for gathering S0 rows), one-hot pooling
    NT_LOC = NLOC // 128
    batch = np.asarray(inputs['batch']).astype(np.int64)
    cnt = np.bincount(batch, minlength=N_GRAPHS).astype(np.float32)
    cnt_recip = (1.0 / np.maximum(cnt, 1.0)).astype(np.float32)
    for pc in cores:
        gids = np.zeros(NLOC, np.int32)
        nn = pc['n_hi'] - pc['n_lo']
        gids[:nn] = np.arange(pc['n_lo'], pc['n_hi'])
        pc['own_gids'] = gids
        oh = np.zeros((NT_LOC, P, N_GRAPHS), np.float32)
        for t in range(NT_LOC):
            for p in range(P):
                n = t * 128 + p
                if n < nn:
                    oh[t, p, batch[pc['n_lo'] + n]] = 1.0
        pc['onehot'] = oh
    return cores, T, NLOC, DUMMY, cnt_recip


def _chunked_idx(arr, T):
    """[T,128] -> [T//CH, 128, CH] so chunk loads are 128B/partition."""
    return np.ascontiguousarray(
        arr.reshape(T // CH, CH, P).transpose(0, 2, 1))


def _weights_host(inputs):
    f = lambda k: np.asarray(inputs[k]).astype(np.float32)
    w = {}
    for li, cin in (('l0', IN_CH), ('l1', HD)):
        Wq, bq = f(f'{li}_Wq'), f(f'{li}_bq')
        Wk, bk = f(f'{li}_Wk'), f(f'{li}_bk')
        Wv, bv = f(f'{li}_Wv'), f(f'{li}_bv')
        We = f(f'{li}_We')[0]          # [128] (EDGE_DIM=1)
        Ws, bs = f(f'{li}_Ws'), f(f'{li}_bs')
        WeH = We.reshape(H, D)
        # qWe[n,h] = q[n,h,:]@We[h,:] -> extra cols in the q table
        wqe = np.zeros((cin, H), np.float32)
        bqe = np.zeros(H, np.float32)
        for h in range(H):
            wqe[:, h] = Wq[:, h * D:(h + 1) * D] @ WeH[h]
            bqe[h] = bq[h * D:(h + 1) * D] @ WeH[h]
        Wq_ext = np.zeros((cin, 136), np.float32)
        Wq_ext[:, :HD] = Wq
        Wq_ext[:, HD:HD + H] = wqe
        bq_ext = np.zeros(136, np.float32)
        bq_ext[:HD] = bq
        bq_ext[HD:HD + H] = bqe
        Wkv = np.concatenate([Wk, Wv], 1)              # [cin, 256]
        bkv = np.concatenate([bk, bv])
        w[f'{li}_wq'] = Wq_ext
        w[f'{li}_bq'] = bq_ext.reshape(1, -1)
        w[f'{li}_wkv'] = Wkv
        w[f'{li}_bkv'] = bkv.reshape(1, -1)
        w[f'{li}_ws'] = Ws
        w[f'{li}_bs'] = bs.reshape(1, -1)
        w[f'{li}_we'] = We.reshape(1, -1)              # [1,128]
    w['cW1'] = f('cW1')
    w['cb1'] = f('cb1').reshape(1, -1)
    w['cW2'] = np.zeros((128, 32), np.float32)
    w['cW2'][:, :N_CLASSES] = f('cW2')
    w['cb2'] = np.zeros((1, 32), np.float32)
    w['cb2'][:, :N_CLASSES] = f('cb2')
    return w


# ---------------- device program ----------------
def _build(T, NLOC, DUMMY):
    nc = bass.Bass()
    dt = F32
    NT_LOC = NLOC // 128
    NT_ALL = NPAD // 128

    # --- I/O ---
    x_in = nc.dram_tensor("x", [NPAD, IN_CH], dt, kind="ExternalInput")
    srcg_in = nc.dram_tensor("srcg", [T // CH, P, CH], I32, kind="ExternalInput")
    srcag_in = nc.dram_tensor("srcag", [T // CH, P, CH], I32, kind="ExternalInput")
    dstg_in = nc.dram_tensor("dstg", [T // CH, P, CH], I32, kind="ExternalInput")
    dstl_in = nc.dram_tensor("dstl", [T // CH, P, CH], I32, kind="ExternalInput")
    eav_in = nc.dram_tensor("eav", [T // CH, P, CH], dt, kind="ExternalInput")
    gids_in = nc.dram_tensor("own_gids", [NLOC // P, P, 1], I32, kind="ExternalInput")
    oneh_in = nc.dram_tensor("onehot", [NT_LOC, P, N_GRAPHS], dt, kind="ExternalInput")
    crec_in = nc.dram_tensor("cnt_recip", [N_GRAPHS, 1], dt, kind="ExternalInput")
    ident_in = nc.dram_tensor("ident", [P, P], dt, kind="ExternalInput")
    ones_in = nc.dram_tensor("onescol", [P, 1], dt, kind="ExternalInput")
    wnames = {}
    for li, cin in (('l0', IN_CH), ('l1', HD)):
        wnames[f'{li}_wq'] = nc.dram_tensor(f"{li}_wq", [cin, 136], dt, kind="ExternalInput")
        wnames[f'{li}_bq'] = nc.dram_tensor(f"{li}_bq", [1, 136], dt, kind="ExternalInput")
        wnames[f'{li}_wkv'] = nc.dram_tensor(f"{li}_wkv", [cin, 256], dt, kind="ExternalInput")
        wnames[f'{li}_bkv'] = nc.dram_tensor(f"{li}_bkv", [1, 256], dt, kind="ExternalInput")
        wnames[f'{li}_ws'] = nc.dram_tensor(f"{li}_ws", [cin, HD], dt, kind="ExternalInput")
        wnames[f'{li}_bs'] = nc.dram_tensor(f"{li}_bs", [1, HD], dt, kind="ExternalInput")
        wnames[f'{li}_we'] = nc.dram_tensor(f"{li}_we", [1, HD], dt, kind="ExternalInput")
    cW1_in = nc.dram_tensor("cW1", [HD, 128], dt, kind="ExternalInput")
    cb1_in = nc.dram_tensor("cb1", [1, 128], dt, kind="ExternalInput")
    cW2_in = nc.dram_tensor("cW2", [128, 32], dt, kind="ExternalInput")
    cb2_in = nc.dram_tensor("cb2", [1, 32], dt, kind="ExternalInput")
    out = nc.dram_tensor("out", [N_GRAPHS, 32], dt, kind="ExternalOutput")

    # --- internal DRAM ---
    tq0 = nc.dram_tensor("tq0", [NPAD, 136], dt)       # global (replicated)
    tkv0 = nc.dram_tensor("tkv0", [NPAD, 256], dt)
    s0 = nc.dram_tensor("s0", [NPAD, HD], dt)
    z0 = nc.dram_tensor("z0", [NLOC, 136], dt)
    z1 = nc.dram_tensor("z1", [NLOC, 136], dt)
    tq1 = nc.dram_tensor("tq1", [NLOC, 136], dt)       # local
    s1 = nc.dram_tensor("s1", [NLOC, HD], dt)
    tkv1_sh = nc.dram_tensor("tkv1_sh", [NLOC, 256], dt)
    tkv1_ag = nc.dram_tensor("tkv1_ag", [NC_CORES * NLOC, 256], dt,
                             addr_space="Shared")
    gsum_b = nc.dram_tensor("gsum_b", [N_GRAPHS, HD], dt)
    gsum_ag = nc.dram_tensor("gsum_ag", [N_GRAPHS, HD], dt, addr_space="Shared")

    IOO = bass.IndirectOffsetOnAxis
    AL = mybir.AluOpType
    ACT = mybir.ActivationFunctionType

    with tile.TileContext(nc) as tc:
        with tc.tile_pool(name="const", bufs=1) as cp:
            ident = cp.tile([P, P], dt)
            nc.sync.dma_start(out=ident[:], in_=ident_in[:])
            onescol = cp.tile([P, 1], dt)
            nc.sync.dma_start(out=onescol[:], in_=ones_in[:])
            zt = cp.tile([P, NLOC * 136 // P], dt)
            nc.vector.memset(zt[:], 0.0)
            nc.sync.dma_start(
                out=z0.ap().rearrange("(a p) w -> p a w", p=P),
                in_=zt[:].rearrange("p (a w) -> p a w", w=136))
            nc.sync.dma_start(
                out=z1.ap().rearrange("(a p) w -> p a w", p=P),
                in_=zt[:].rearrange("p (a w) -> p a w", w=136))

            # ---------- node phase, layer 0 (replicated, all nodes) ----------
            with tc.tile_pool(name="np0", bufs=3) as sp, \
                 tc.tile_pool(name="np0w", bufs=1) as wp, \
                 tc.tile_pool(name="np0p", bufs=2, space="PSUM") as pp:
                wq0 = wp.tile([IN_CH, 136], dt)
                nc.sync.dma_start(out=wq0[:], in_=wnames['l0_wq'][:])
                wkv0 = wp.tile([IN_CH, 256], dt)
                nc.sync.dma_start(out=wkv0[:], in_=wnames['l0_wkv'][:])
                ws0 = wp.tile([IN_CH, HD], dt)
                nc.sync.dma_start(out=ws0[:], in_=wnames['l0_ws'][:])
                bq0 = wp.tile([P, 136], dt)
                nc.sync.dma_start(out=bq0[:], in_=wnames['l0_bq'][:].to_broadcast([P, 136]))
                bkv0 = wp.tile([P, 256], dt)
                nc.sync.dma_start(out=bkv0[:], in_=wnames['l0_bkv'][:].to_broadcast([P, 256]))
                bs0 = wp.tile([P, HD], dt)
                nc.sync.dma_start(out=bs0[:], in_=wnames['l0_bs'][:].to_broadcast([P, HD]))
                for t in range(NT_ALL):
                    xt = sp.tile([P, IN_CH], dt, tag="xt")
                    nc.sync.dma_start(out=xt[:], in_=x_in[t * P:(t + 1) * P, :])
                    xT_p = pp.tile([IN_CH, P], dt, tag="xT", space="PSUM")
                    nc.tensor.transpose(out=xT_p[:], in_=xt[:], identity=ident[:])
                    xT = sp.tile([IN_CH, P], dt, tag="xTs")
                    nc.vector.tensor_copy(out=xT[:], in_=xT_p[:])
                    pq = pp.tile([P, 136], dt, tag="pq", space="PSUM")
                    nc.tensor.matmul(pq[:], lhsT=xT[:], rhs=wq0[:], start=True, stop=True)
                    oq = sp.tile([P, 136], dt, tag="oq")
                    nc.vector.tensor_tensor(out=oq[:], in0=pq[:], in1=bq0[:], op=AL.add)
                    nc.sync.dma_start(out=tq0[t * P:(t + 1) * P, :], in_=oq[:])
                    pkv = pp.tile([P, 256], dt, tag="pkv", space="PSUM")
                    nc.tensor.matmul(pkv[:], lhsT=xT[:], rhs=wkv0[:], start=True, stop=True)
                    okv = sp.tile([P, 256], dt, tag="okv")
                    nc.vector.tensor_tensor(out=okv[:], in0=pkv[:], in1=bkv0[:], op=AL.add)
                    nc.sync.dma_start(out=tkv0[t * P:(t + 1) * P, :], in_=okv[:])
                    ps = pp.tile([P, HD], dt, tag="ps", space="PSUM")
                    nc.tensor.matmul(ps[:], lhsT=xT[:], rhs=ws0[:], start=True, stop=True)
                    os_ = sp.tile([P, HD], dt, tag="os")
                    nc.vector.tensor_tensor(out=os_[:], in0=ps[:], in1=bs0[:], op=AL.add)
                    nc.sync.dma_start(out=s0[t * P:(t + 1) * P, :], in_=os_[:])

            # ---------- edge pass (shared for both layers) ----------
            def edge_pass(tag, tq_tab, kv_tab, z_tab, src_in, dstq_in):
                with tc.tile_pool(name=f"eidx{tag}", bufs=2) as ep, \
                     tc.tile_pool(name=f"ebuf{tag}", bufs=3) as eb, \
                     tc.tile_pool(name=f"epp{tag}", bufs=2, space="PSUM") as pp:
                    for ch in range(T // CH):
                        srcT = ep.tile([P, CH], I32, tag="srcT")
                        nc.sync.dma_start(out=srcT[:], in_=src_in[ch])
                        dqT = ep.tile([P, CH], I32, tag="dqT")
                        nc.sync.dma_start(out=dqT[:], in_=dstq_in[ch])
                        dlT = ep.tile([P, CH], I32, tag="dlT")
                        nc.sync.dma_start(out=dlT[:], in_=dstl_in[ch])
                        eaT = ep.tile([P, CH], dt, tag="eaT")
                        nc.sync.dma_start(out=eaT[:], in_=eav_in[ch])
                        dlF = ep.tile([P, CH], dt, tag="dlF")
                        nc.vector.tensor_copy(out=dlF[:], in_=dlT[:])
                        for b in range(CH // B):
                            t0 = b * B
                            kvB = eb.tile([P, B, 256], dt, tag="kvB")
                            qB = eb.tile([P, B, 136], dt, tag="qB")
                            for j in range(B):
                                nc.gpsimd.indirect_dma_start(
                                    out=kvB[:, j, :], out_offset=None, in_=kv_tab[:],
                                    in_offset=IOO(ap=srcT[:, t0 + j:t0 + j + 1], axis=0))
                                nc.gpsimd.indirect_dma_start(
                                    out=qB[:, j, :], out_offset=None, in_=tq_tab[:],
                                    in_offset=IOO(ap=dqT[:, t0 + j:t0 + j + 1], axis=0))
                            # Sel build
                            tp = pp.tile([P, B, P], dt, tag="tp", space="PSUM")
                            for j in range(B):
                                nc.tensor.transpose(
                                    out=tp[:, j, :],
                                    in_=dlF[:, t0 + j:t0 + j + 1].to_broadcast([P, P]),
                                    identity=ident[:])
                            tsb = eb.tile([P, B, P], dt, tag="tsb")
                            nc.vector.tensor_copy(out=tsb[:], in_=tp[:])
                            sel = eb.tile([P, B, P], dt, tag="sel")
                            nc.vector.tensor_tensor(
                                out=sel[:],
                                in0=dlF[:, t0:t0 + B, None].to_broadcast([P, B, P]),
                                in1=tsb[:], op=AL.is_equal)
                            # logits / ex
                            prod = eb.tile([P, B, HD], dt, tag="prod")
                            nc.vector.tensor_tensor(
                                out=prod[:], in0=qB[:, :, 0:HD], in1=kvB[:, :, 0:HD],
                                op=AL.mult)
                            lg = eb.tile([P, B, H, 1], dt, tag="lg")
                            nc.vector.tensor_reduce(
                                out=lg[:],
                                in_=prod[:].rearrange("p b (h d) -> p b h d", h=H),
                                op=AL.add, axis=mybir.AxisListType.X)
                            eaq = eb.tile([P, B, H, 1], dt, tag="eaq")
                            nc.vector.tensor_tensor(
                                out=eaq[:], in0=qB[:, :, HD:HD + H, None],
                                in1=eaT[:, t0:t0 + B, None, None].to_broadcast([P, B, H, 1]),
                                op=AL.mult)
                            nc.vector.tensor_tensor(
                                out=lg[:], in0=lg[:], in1=eaq[:], op=AL.add)
                            ex = eb.tile([P, B, H, 1], dt, tag="ex")
                            nc.scalar.activation(ex[:], lg[:], ACT.Exp, scale=0.125)
                            # SelEx per head
                            selex = eb.tile([P, B, H, P], dt, tag="selex")
                            nc.vector.tensor_tensor(
                                out=selex[:],
                                in0=sel[:, :, None, :].to_broadcast([P, B, H, P]),
                                in1=ex[:].to_broadcast([P, B, H, P]),
                                op=AL.mult)
                            # onesea rhs [P, B, 2]
                            onea = eb.tile([P, B, 2], dt, tag="onea")
                            nc.vector.memset(onea[:, :, 0:1], 1.0)
                            nc.vector.tensor_copy(
                                out=onea[:, :, 1:2], in_=eaT[:, t0:t0 + B, None])
                            # combine matmuls
                            pv = pp.tile([P, B, HD], dt, tag="pv", space="PSUM")
                            psm = pp.tile([P, B, 4], dt, tag="psm", space="PSUM")
                            for j in range(B):
                                for h in range(H):
                                    nc.tensor.matmul(
                                        pv[:, j, h * D:(h + 1) * D],
                                        lhsT=selex[:, j, h, :],
                                        rhs=kvB[:, j, HD + h * D:HD + (h + 1) * D],
                                        start=True, stop=True)
                                    nc.tensor.matmul(
                                        psm[:, j, 2 * h:2 * h + 2],
                                        lhsT=selex[:, j, h, :],
                                        rhs=onea[:, j, :],
                                        start=True, stop=True)
                            sums = eb.tile([P, B, 136], dt, tag="sums")
                            nc.vector.tensor_copy(out=sums[:, :, 0:HD], in_=pv[:])
                            nc.vector.tensor_copy(out=sums[:, :, HD:HD + 4], in_=psm[:])
                            for j in range(B):
                                nc.gpsimd.indirect_dma_start(
                                    out=z_tab[:],
                                    out_offset=IOO(ap=dlT[:, t0 + j:t0 + j + 1], axis=0),
                                    in_=sums[:, j, :], in_offset=None)

            edge_pass("0", tq0, tkv0, z0,
                      [srcg_in[ch] for ch in range(T // CH)],
                      [dstg_in[ch] for ch in range(T // CH)])

            # ---------- node phase, layer 1 (own shard only) ----------
            with tc.tile_pool(name="np1", bufs=3) as sp, \
                 tc.tile_pool(name="np1w", bufs=1) as wp, \
                 tc.tile_pool(name="np1p", bufs=2, space="PSUM") as pp:
                wq1 = wp.tile([HD, 136], dt)
                nc.sync.dma_start(out=wq1[:], in_=wnames['l1_wq'][:])
                wkv1 = wp.tile([HD, 256], dt)
                nc.sync.dma_start(out=wkv1[:], in_=wnames['l1_wkv'][:])
                ws1 = wp.tile([HD, HD], dt)
                nc.sync.dma_start(out=ws1[:], in_=wnames['l1_ws'][:])
                bq1 = wp.tile([P, 136], dt)
                nc.sync.dma_start(out=bq1[:], in_=wnames['l1_bq'][:].to_broadcast([P, 136]))
                bkv1 = wp.tile([P, 256], dt)
                nc.sync.dma_start(out=bkv1[:], in_=wnames['l1_bkv'][:].to_broadcast([P, 256]))
                bs1 = wp.tile([P, HD], dt)
                nc.sync.dma_start(out=bs1[:], in_=wnames['l1_bs'][:].to_broadcast([P, HD]))
                we0b = wp.tile([P, HD], dt)
                nc.sync.dma_start(out=we0b[:], in_=wnames['l0_we'][:].to_broadcast([P, HD]))
                gidT = wp.tile([P, NLOC // P], I32)
                nc.sync.dma_start(out=gidT[:],
                                  in_=gids_in.ap().rearrange("a p o -> p (a o)"))
                for t in range(NT_LOC):
                    zr = sp.tile([P, 136], dt, tag="zr")
                    nc.sync.dma_start(out=zr[:], in_=z0[t * P:(t + 1) * P, :])
                    s0r = sp.tile([P, HD], dt, tag="s0r")
                    nc.gpsimd.indirect_dma_start(
                        out=s0r[:], out_offset=None, in_=s0[:],
                        in_offset=IOO(ap=gidT[:, t:t + 1], axis=0))
                    h1 = sp.tile([P, HD], dt, tag="h1")
                    _agg_relu(nc, sp, h1, zr, s0r, we0b)
                    hT_p = pp.tile([P, P], dt, tag="hT", space="PSUM")
                    nc.tensor.transpose(out=hT_p[:], in_=h1[:], identity=ident[:])
                    hT = sp.tile([P, P], dt, tag="hTs")
                    nc.vector.tensor_copy(out=hT[:], in_=hT_p[:])
                    pq = pp.tile([P, 136], dt, tag="pq1", space="PSUM")
                    nc.tensor.matmul(pq[:], lhsT=hT[:], rhs=wq1[:], start=True, stop=True)
                    oq = sp.tile([P, 136], dt, tag="oq1")
                    nc.vector.tensor_tensor(out=oq[:], in0=pq[:], in1=bq1[:], op=AL.add)
                    nc.sync.dma_start(out=tq1[t * P:(t + 1) * P, :], in_=oq[:])
                    pkv = pp.tile([P, 256], dt, tag="pkv1", space="PSUM")
                    nc.tensor.matmul(pkv[:], lhsT=hT[:], rhs=wkv1[:], start=True, stop=True)
                    okv = sp.tile([P, 256], dt, tag="okv1")
                    nc.vector.tensor_tensor(out=okv[:], in0=pkv[:], in1=bkv1[:], op=AL.add)
                    nc.sync.dma_start(out=tkv1_sh[t * P:(t + 1) * P, :], in_=okv[:])
                    ps = pp.tile([P, HD], dt, tag="ps1", space="PSUM")
                    nc.tensor.matmul(ps[:], lhsT=hT[:], rhs=ws1[:], start=True, stop=True)
                    os_ = sp.tile([P, HD], dt, tag="os1")
                    nc.vector.tensor_tensor(out=os_[:], in0=ps[:], in1=bs1[:], op=AL.add)
                    nc.sync.dma_start(out=s1[t * P:(t + 1) * P, :], in_=os_[:])

            nc.gpsimd.collective_compute(
                "AllGather", AL.bypass,
                replica_groups=[list(range(NC_CORES))],
                ins=[tkv1_sh.ap().opt()], outs=[tkv1_ag.ap().opt()])

            edge_pass("1", tq1, tkv1_ag, z1,
                      [srcag_in[ch] for ch in range(T // CH)],
                      [dstl_in[ch] for ch in range(T // CH)])

            # ---------- final: h2, pooling, MLP ----------
            with tc.tile_pool(name="fp", bufs=3) as sp, \
                 tc.tile_pool(name="fpw", bufs=1) as wp, \
                 tc.tile_pool(name="fpp", bufs=1, space="PSUM") as pp:
                we1b = wp.tile([P, HD], dt)
                nc.sync.dma_start(out=we1b[:], in_=wnames['l1_we'][:].to_broadcast([P, HD]))
                pg = pp.tile([N_GRAPHS, HD], dt, space="PSUM")
                for t in range(NT_LOC):
                    zr = sp.tile([P, 136], dt, tag="zr2")
                    nc.sync.dma_start(out=zr[:], in_=z1[t * P:(t + 1) * P, :])
                    s1r = sp.tile([P, HD], dt, tag="s1r")
                    nc.sync.dma_start(out=s1r[:], in_=s1[t * P:(t + 1) * P, :])
                    h2 = sp.tile([P, HD], dt, tag="h2")
                    _agg_relu(nc, sp, h2, zr, s1r, we1b)
                    oh = sp.tile([P, N_GRAPHS], dt, tag="oh")
                    nc.sync.dma_start(out=oh[:], in_=oneh_in[t])
                    nc.tensor.matmul(pg[:], lhsT=oh[:], rhs=h2[:],
                                     start=(t == 0), stop=(t == NT_LOC - 1))
                gs = wp.tile([N_GRAPHS, HD], dt)
                nc.vector.tensor_copy(out=gs[:], in_=pg[:])
                nc.sync.dma_start(out=gsum_b[:], in_=gs[:])
            nc.gpsimd.collective_compute(
                "AllReduce", AL.add,
                replica_groups=[list(range(NC_CORES))],
                ins=[gsum_b.ap().opt()], outs=[gsum_ag.ap().opt()])
            with tc.tile_pool(name="mlp", bufs=1) as sp, \
                 tc.tile_pool(name="mlpp", bufs=2, space="PSUM") as pp:
                g = sp.tile([N_GRAPHS, HD], dt)
                nc.sync.dma_start(out=g[:], in_=gsum_ag[:])
                cr = sp.tile([N_GRAPHS, 1], dt)
                nc.sync.dma_start(out=cr[:], in_=crec_in[:])
                nc.vector.tensor_scalar(out=g[:], in0=g[:], scalar1=cr[:],
                                        scalar2=None, op0=AL.mult)
                w1 = sp.tile([HD, 128], dt)
                nc.sync.dma_start(out=w1[:], in_=cW1_in[:])
                b1 = sp.tile([N_GRAPHS, 128], dt)
                nc.sync.dma_start(out=b1[:], in_=cb1_in[:].to_broadcast([N_GRAPHS, 128]))
                w2 = sp.tile([128, 32], dt)
                nc.sync.dma_start(out=w2[:], in_=cW2_in[:])
                b2 = sp.tile([N_GRAPHS, 32], dt)
                nc.sync.dma_start(out=b2[:], in_=cb2_in[:].to_broadcast([N_GRAPHS, 32]))
                gT_p = pp.tile([HD, N_GRAPHS], dt, tag="gT", space="PSUM")
                nc.tensor.transpose(out=gT_p[:], in_=g[:], identity=ident[:N_GRAPHS, :N_GRAPHS])
                gT = sp.tile([HD, N_GRAPHS], dt)
                nc.vector.tensor_copy(out=gT[:], in_=gT_p[:])
                p1 = pp.tile([N_GRAPHS, 128], dt, tag="p1", space="PSUM")
                nc.tensor.matmul(p1[:], lhsT=gT[:], rhs=w1[:], start=True, stop=True)
                y1 = sp.tile([N_GRAPHS, 128], dt)
                nc.vector.tensor_tensor(out=y1[:], in0=p1[:], in1=b1[:], op=AL.add)
                nc.scalar.activation(y1[:], y1[:], ACT.Relu)
                y1T_p = pp.tile([128, N_GRAPHS], dt, tag="y1T", space="PSUM")
                nc.tensor.transpose(out=y1T_p[:], in_=y1[:], identity=ident[:N_GRAPHS, :N_GRAPHS])
                y1T = sp.tile([128, N_GRAPHS], dt)
                nc.vector.tensor_copy(out=y1T[:], in_=y1T_p[:])
                p2 = pp.tile([N_GRAPHS, 32], dt, tag="p2", space="PSUM")
                nc.tensor.matmul(p2[:], lhsT=y1T[:], rhs=w2[:], start=True, stop=True)
                y2 = sp.tile([N_GRAPHS, 32], dt)
                nc.vector.tensor_tensor(out=y2[:], in0=p2[:], in1=b2[:], op=AL.add)
                nc.sync.dma_start(out=out[:], in_=y2[:])

    _legalize_waits(nc)
    return nc


def _agg_relu(nc, sp, h_out, zr, skip, we_b):
    """h = relu(Zv/(Zex+eps) + (Zexea/(Zex+eps))*We + skip).
    Z layout: [v(128) | ex0, exea0, ex1, exea1]."""
    dt = F32
    AL = mybir.AluOpType
    ACT = mybir.ActivationFunctionType
    zex = zr[:, HD:HD + 4].rearrange("p (h two) -> p h two", h=H)[:, :, 0:1]   # [P,H,1]
    zea = zr[:, HD:HD + 4].rearrange("p (h two) -> p h two", h=H)[:, :, 1:2]
    r = sp.tile([P, H, 1], dt, tag="rcp")
    nc.vector.tensor_scalar(out=r[:], in0=zex, scalar1=1e-16, scalar2=None, op0=AL.add)
    nc.vector.reciprocal(out=r[:], in_=r[:])
    rea = sp.tile([P, H, 1], dt, tag="rea")
    nc.vector.tensor_tensor(out=rea[:], in0=zea, in1=r[:], op=AL.mult)
    nc.vector.tensor_tensor(
        out=h_out[:].rearrange("p (h d) -> p h d", h=H),
        in0=zr[:, 0:HD].rearrange("p (h d) -> p h d", h=H),
        in1=r[:].to_broadcast([P, H, D]), op=AL.mult)
    wterm = sp.tile([P, HD], dt, tag="wterm")
    nc.vector.tensor_tensor(
        out=wterm[:].rearrange("p (h d) -> p h d", h=H),
        in0=we_b[:].rearrange("p (h d) -> p h d", h=H),
        in1=rea[:].to_broadcast([P, H, D]), op=AL.mult)
    nc.vector.tensor_tensor(out=h_out[:], in0=h_out[:], in1=wterm[:], op=AL.add)
    nc.vector.tensor_tensor(out=h_out[:], in0=h_out[:], in1=skip[:], op=AL.add)
    nc.scalar.activation(h_out[:], h_out[:], ACT.Relu)


_CACHE = {}


def kernel(**inputs):
    cores, T, NLOC, DUMMY, cnt_recip = _host_prep(inputs)
    w = _weights_host(inputs)

    key = (T, NLOC)
    if key not in _CACHE:
        _CACHE[key] = _build(T, NLOC, DUMMY)
    nc = _CACHE[key]

    xpad = np.zeros((NPAD, IN_CH), np.float32)
    xpad[:N_NODES] = np.asarray(inputs['x']).astype(np.float32)
    common = dict(
        x=xpad,
        cnt_recip=cnt_recip.reshape(N_GRAPHS, 1),
        ident=np.eye(P, dtype=np.float32),
        onescol=np.ones((P, 1), np.float32),
        cW1=w['cW1'], cb1=w['cb1'], cW2=w['cW2'], cb2=w['cb2'],
    )
    for li in ('l0', 'l1'):
        for nm in ('wq', 'bq', 'wkv', 'bkv', 'ws', 'bs', 'we'):
            common[f'{li}_{nm}'] = w[f'{li}_{nm}']

    in_maps = []
    for pc in cores:
        m = dict(common)
        m['srcg'] = _chunked_idx(pc['srcg'], T)
        m['srcag'] = _chunked_idx(pc['srcag'], T)
        m['dstg'] = _chunked_idx(pc['dstg'], T)
        m['dstl'] = _chunked_idx(pc['dstl'], T)
        m['eav'] = _chunked_idx(pc['eav'], T)
        m['own_gids'] = pc['own_gids'].reshape(NLOC // P, P, 1)
        m['onehot'] = pc['onehot']
        in_maps.append(m)

    res = run_bass_kernel_spmd(nc, in_maps, core_ids=list(range(NC_CORES)))
    out = np.asarray(res.results[0]['out'])[:, :N_CLASSES]
    return out.astype(np.float32)


if __name__ == "__main__":
    import reference  # only for standalone self-test; harness calls kernel()
    inp = {k: np.asarray(v) for k, v in reference.setup_inputs().items()}
    got = kernel(**inp)
    exp = np.asarray(reference.reference(**inp))
    err = np.abs(got - exp).max() / (np.abs(exp).max() + 1e-12)
    print("rel err:", err)


# revision 6
# speedup vs baseline: 1.0308x; 1.0308x over previous
"""Trainium2 Bass kernel for 2-layer TransformerConv GNN + mean-pool head.

Strategy (SPMD over 8 cores):
  * Host sorts edges by dst, shards by dst-range (edge-balanced, snapped to
    node boundaries). Each core owns a contiguous dst node range; its Z
    (softmax numerator/denominator sums per node) is complete locally.
  * Edge tiles of 128 edges, dst-ALIGNED (no dst straddles a tile), so the
    per-tile selection-matrix matmul produces COMPLETE per-dst sums and a
    bypass indirect scatter (duplicate rows carry identical values) needs
    no cross-tile accumulation.
  * logits = q[dst]·k[src] + ea*(q[dst]·We)  (qWe precomputed per node);
    softmax without max-subtraction (logits are O(1); exp is safe);
    alpha folding: Z = [Σ ex*v | Σ ex | Σ ex*ea] per node, divided at node
    level, so no per-edge denominator gather.
  * Layer-1 node tables: each core computes its shard from its Z, then one
    AllGather shares the (k|v) table; q/skip tables stay local (dst-local).
  * Final: per-core partial graph sums via accumulated one-hot matmuls,
    one AllReduce [64,128], replicated tiny MLP head.
"""
import sys
sys.path.insert(0, '/opt/trn_rl_repo')
import copy as _copy
import numpy as np

import concourse.bass as bass
import concourse.tile as tile
from concourse import mybir
from concourse.tile_rust import add_dep_helper
from concourse.bass_utils import run_bass_kernel_spmd
import bass_rust as _bass_rust

# ---------------- problem constants (hardcoded per spec) ----------------
H, D, HD = 2, 64, 128
N_NODES, N_EDGES, N_GRAPHS = 50000, 800000, 64
IN_CH, EDGE_DIM, N_CLASSES = 2, 1, 30
NC_CORES = 8
P = 128
B = 4           # edge tiles per batch
CH = 32         # edge tiles per index-chunk load
F32 = mybir.dt.float32
I32 = mybir.dt.int32

NPAD = ((N_NODES + 127) // 128) * 128          # 50048


# ---------------- walrus sync-wait legalization (this toolchain caps
# embedded waits at 1 per instruction, 2 for EventSemaphore) --------------
def _legalize_waits(nc):
    m = nc.m
    new_module = _copy.replace(m, functions=[])
    for function in m.functions:
        nf = _copy.replace(function, blocks=[])
        nf.set_allocations_from_list(function.allocations)
        for block in function.blocks:
            insts = []
            for inst in block.instructions:
                si = inst.sync_info
                waits = list(si.on_wait) if si and si.on_wait else []
                cap = 2 if isinstance(inst, mybir.InstEventSemaphore) else 1
                if len(waits) > cap:
                    extra, keep = waits[:-cap], waits[-cap:]
                    for i in range(0, len(extra), 2):
                        insts.append(mybir.InstEventSemaphore(
                            name=f"{inst.name}-wait{i}",
                            engine=inst.engine,
                            sync_info=_bass_rust.SyncInfo(
                                on_wait=extra[i:i + 2], on_update=[]),
                        ))
                    inst.sync_info = _bass_rust.SyncInfo(
                        on_wait=keep,
                        on_update=list(si.on_update) if si.on_update else [])
                insts.append(inst)
            nf.blocks.append(_copy.replace(block, instructions=insts))
        new_module.functions.append(nf)
    nc.m = new_module


# ---------------- host-side sharding ----------------
def _host_prep(inputs):
    src = np.asarray(inputs['edge_index'][0]).astype(np.int64)
    dst = np.asarray(inputs['edge_index'][1]).astype(np.int64)
    ea = np.asarray(inputs['edge_attr'][:, 0]).astype(np.float32)
    order = np.argsort(dst, kind='stable')
    src, dst, ea = src[order], dst[order], ea[order]

    # core boundaries: ~E/8 edges each, snapped so no dst node straddles
    cuts = [0]
    for c in range(1, NC_CORES):
        e = c * N_EDGES // NC_CORES
        while e < N_EDGES and e > 0 and dst[e] == dst[e - 1]:
            e += 1
        cuts.append(e)
    cuts.append(N_EDGES)

    per_core = []
    for c in range(NC_CORES):
        lo, hi = cuts[c], cuts[c + 1]
        s, d, a = src[lo:hi], dst[lo:hi], ea[lo:hi]
        n_lo = int(d[0]) if hi > lo else 0
        n_hi = int(d[-1]) + 1 if hi > lo else 1
        # dst-aligned tiles of <=128 edges (greedy pack whole-dst groups)
        bnd = np.flatnonzero(np.diff(d)) + 1
        groups = np.split(np.arange(hi - lo), bnd)
        tiles, cur, cnt = [], [], 0
        for g in groups:
            if cnt + len(g) > P:
                tiles.append(np.concatenate(cur))
                cur, cnt = [], 0
            assert len(g) <= P, "node degree > 128 unsupported"
            cur.append(g)
            cnt += len(g)
        if cur:
            tiles.append(np.concatenate(cur))
        per_core.append(dict(s=s, d=d, a=a, n_lo=n_lo, n_hi=n_hi, tiles=tiles))

    T = max(len(pc['tiles']) for pc in per_core)
    T = ((T + CH - 1) // CH) * CH
    NLOC = max(pc['n_hi'] - pc['n_lo'] for pc in per_core)
    NLOC = ((NLOC + 2 + 127) // 128) * 128          # +dummy row, 128-mult
    DUMMY = NLOC - 1

    cores = []
    for pc in per_core:
        srcg = np.zeros((T, P), np.int32)
        dstg = np.zeros((T, P), np.int32)
        dstl = np.full((T, P), DUMMY, np.int32)
        eav = np.zeros((T, P), np.float32)
        for t, idx in enumerate(pc['tiles']):
            k = len(idx)
            srcg[t, :k] = pc['s'][idx]
            dstg[t, :k] = pc['d'][idx]
            dstl[t, :k] = pc['d'][idx] - pc['n_lo']
        for t, idx in enumerate(pc['tiles']):
            eav[t, :len(idx)] = pc['a'][idx]
        cores.append(dict(srcg=srcg, dstg=dstg, dstl=dstl, eav=eav,
                          n_lo=pc['n_lo'], n_hi=pc['n_hi']))

    # AllGather-space src index: core c's rows live at c*NLOC + (n - n_lo[c])
    node_core = np.zeros(N_NODES, np.int64)
    node_loc = np.zeros(N_NODES, np.int64)
    for c, pc in enumerate(cores):
        node_core[pc['n_lo']:pc['n_hi']] = c
        node_loc[pc['n_lo']:pc['n_hi']] = (
            np.arange(pc['n_lo'], pc['n_hi']) - pc['n_lo'] + c * NLOC)
    for c, pc in enumerate(cores):
        sg = pc['srcg']
        pc['srcag'] = node_loc[sg].astype(np.int32)

    # per-core own-node global ids (for gathering S0 rows), one-hot pooling
    NT_LOC = NLOC // 128
    batch = np.asarray(inputs['batch']).astype(np.int64)
    cnt = np.bincount(batch, minlength=N_GRAPHS).astype(np.float32)
    cnt_recip = (1.0 / np.maximum(cnt, 1.0)).astype(np.float32)
    for pc in cores:
        gids = np.zeros(NLOC, np.int32)
        nn = pc['n_hi'] - pc['n_lo']
        gids[:nn] = np.arange(pc['n_lo'], pc['n_hi'])
        pc['own_gids'] = gids
        oh = np.zeros((NT_LOC, P, N_GRAPHS), np.float32)
        for t in range(NT_LOC):
            for p in range(P):
                n = t * 128 + p
                if n < nn:
                    oh[t, p, batch[pc['n_lo'] + n]] = 1.0
        pc['onehot'] = oh
    return cores, T, NLOC, DUMMY, cnt_recip


def _chunked_idx(arr, T):
    """[T,128] -> [T//CH, 128, CH] so chunk loads are 128B/partition."""
    return np.ascontiguousarray(
        arr.reshape(T // CH, CH, P).transpose(0, 2, 1))


def _weights_host(inputs):
    f = lambda k: np.asarray(inputs[k]).astype(np.float32)
    w = {}
    for li, cin in (('l0', IN_CH), ('l1', HD)):
        Wq, bq = f(f'{li}_Wq'), f(f'{li}_bq')
        Wk, bk = f(f'{li}_Wk'), f(f'{li}_bk')
        Wv, bv = f(f'{li}_Wv'), f(f'{li}_bv')
        We = f(f'{li}_We')[0]          # [128] (EDGE_DIM=1)
        Ws, bs = f(f'{li}_Ws'), f(f'{li}_bs')
        WeH = We.reshape(H, D)
        # qWe[n,h] = q[n,h,:]@We[h,:] -> extra cols in the q table
        wqe = np.zeros((cin, H), np.float32)
        bqe = np.zeros(H, np.float32)
        for h in range(H):
            wqe[:, h] = Wq[:, h * D:(h + 1) * D] @ WeH[h]
            bqe[h] = bq[h * D:(h + 1) * D] @ WeH[h]
        Wq_ext = np.zeros((cin, 136), np.float32)
        Wq_ext[:, :HD] = Wq
        Wq_ext[:, HD:HD + H] = wqe
        bq_ext = np.zeros(136, np.float32)
        bq_ext[:HD] = bq
        bq_ext[HD:HD + H] = bqe
        Wkv = np.concatenate([Wk, Wv], 1)              # [cin, 256]
        bkv = np.concatenate([bk, bv])
        w[f'{li}_wq'] = Wq_ext
        w[f'{li}_bq'] = bq_ext.reshape(1, -1)
        w[f'{li}_wkv'] = Wkv
        w[f'{li}_bkv'] = bkv.reshape(1, -1)
        w[f'{li}_ws'] = Ws
        w[f'{li}_bs'] = bs.reshape(1, -1)
        w[f'{li}_we'] = We.reshape(1, -1)              # [1,128]
    w['cW1'] = f('cW1')
    w['cb1'] = f('cb1').reshape(1, -1)
    w['cW2'] = np.zeros((128, 32), np.float32)
    w['cW2'][:, :N_CLASSES] = f('cW2')
    w['cb2'] = np.zeros((1, 32), np.float32)
    w['cb2'][:, :N_CLASSES] = f('cb2')
    return w


# ---------------- device program ----------------
def _build(T, NLOC, DUMMY):
    nc = bass.Bass()
    dt = F32
    NT_LOC = NLOC // 128
    NT_ALL = NPAD // 128

    # --- I/O ---
    x_in = nc.dram_tensor("x", [NPAD, IN_CH], dt, kind="ExternalInput")
    srcg_in = nc.dram_tensor("srcg", [T // CH, P, CH], I32, kind="ExternalInput")
    srcag_in = nc.dram_tensor("srcag", [T // CH, P, CH], I32, kind="ExternalInput")
    dstg_in = nc.dram_tensor("dstg", [T // CH, P, CH], I32, kind="ExternalInput")
    dstl_in = nc.dram_tensor("dstl", [T // CH, P, CH], I32, kind="ExternalInput")
    eav_in = nc.dram_tensor("eav", [T // CH, P, CH], dt, kind="ExternalInput")
    gids_in = nc.dram_tensor("own_gids", [NLOC // P, P, 1], I32, kind="ExternalInput")
    oneh_in = nc.dram_tensor("onehot", [NT_LOC, P, N_GRAPHS], dt, kind="ExternalInput")
    crec_in = nc.dram_tensor("cnt_recip", [N_GRAPHS, 1], dt, kind="ExternalInput")
    ident_in = nc.dram_tensor("ident", [P, P], dt, kind="ExternalInput")
    ones_in = nc.dram_tensor("onescol", [P, 1], dt, kind="ExternalInput")
    wnames = {}
    for li, cin in (('l0', IN_CH), ('l1', HD)):
        wnames[f'{li}_wq'] = nc.dram_tensor(f"{li}_wq", [cin, 136], dt, kind="ExternalInput")
        wnames[f'{li}_bq'] = nc.dram_tensor(f"{li}_bq", [1, 136], dt, kind="ExternalInput")
        wnames[f'{li}_wkv'] = nc.dram_tensor(f"{li}_wkv", [cin, 256], dt, kind="ExternalInput")
        wnames[f'{li}_bkv'] = nc.dram_tensor(f"{li}_bkv", [1, 256], dt, kind="ExternalInput")
        wnames[f'{li}_ws'] = nc.dram_tensor(f"{li}_ws", [cin, HD], dt, kind="ExternalInput")
        wnames[f'{li}_bs'] = nc.dram_tensor(f"{li}_bs", [1, HD], dt, kind="ExternalInput")
        wnames[f'{li}_we'] = nc.dram_tensor(f"{li}_we", [1, HD], dt, kind="ExternalInput")
    cW1_in = nc.dram_tensor("cW1", [HD, 128], dt, kind="ExternalInput")
    cb1_in = nc.dram_tensor("cb1", [1, 128], dt, kind="ExternalInput")
    cW2_in = nc.dram_tensor("cW2", [128, 32], dt, kind="ExternalInput")
    cb2_in = nc.dram_tensor("cb2", [1, 32], dt, kind="ExternalInput")
    out = nc.dram_tensor("out", [N_GRAPHS, 32], dt, kind="ExternalOutput")

    # --- internal DRAM ---
    tq0 = nc.dram_tensor("tq0", [NPAD, 136], dt)       # global (replicated)
    tkv0 = nc.dram_tensor("tkv0", [NPAD, 256], dt)
    s0 = nc.dram_tensor("s0", [NPAD, HD], dt)
    z0 = nc.dram_tensor("z0", [NLOC, 136], dt)
    z1 = nc.dram_tensor("z1", [NLOC, 136], dt)
    tq1 = nc.dram_tensor("tq1", [NLOC, 136], dt)       # local
    s1 = nc.dram_tensor("s1", [NLOC, HD], dt)
    tkv1_sh = nc.dram_tensor("tkv1_sh", [NLOC, 256], dt)
    tkv1_ag = nc.dram_tensor("tkv1_ag", [NC_CORES * NLOC, 256], dt,
                             addr_space="Shared")
    gsum_b = nc.dram_tensor("gsum_b", [N_GRAPHS, HD], dt)
    gsum_ag = nc.dram_tensor("gsum_ag", [N_GRAPHS, HD], dt, addr_space="Shared")

    IOO = bass.IndirectOffsetOnAxis
    AL = mybir.AluOpType
    ACT = mybir.ActivationFunctionType

    with tile.TileContext(nc) as tc:
        with tc.tile_pool(name="const", bufs=1) as cp:
            ident = cp.tile([P, P], dt)
            nc.sync.dma_start(out=ident[:], in_=ident_in[:])
            onescol = cp.tile([P, 1], dt)
            nc.sync.dma_start(out=onescol[:], in_=ones_in[:])
            zt = cp.tile([P, NLOC * 136 // P], dt)
            nc.vector.memset(zt[:], 0.0)
            nc.sync.dma_start(
                out=z0.ap().rearrange("(a p) w -> p a w", p=P),
                in_=zt[:].rearrange("p (a w) -> p a w", w=136))
            nc.sync.dma_start(
                out=z1.ap().rearrange("(a p) w -> p a w", p=P),
                in_=zt[:].rearrange("p (a w) -> p a w", w=136))

            # ---------- node phase, layer 0 (replicated, all nodes) ----------
            with tc.tile_pool(name="np0", bufs=3) as sp, \
                 tc.tile_pool(name="np0w", bufs=1) as wp, \
                 tc.tile_pool(name="np0p", bufs=2, space="PSUM") as pp:
                wq0 = wp.tile([IN_CH, 136], dt)
                nc.sync.dma_start(out=wq0[:], in_=wnames['l0_wq'][:])
                wkv0 = wp.tile([IN_CH, 256], dt)
                nc.sync.dma_start(out=wkv0[:], in_=wnames['l0_wkv'][:])
                ws0 = wp.tile([IN_CH, HD], dt)
                nc.sync.dma_start(out=ws0[:], in_=wnames['l0_ws'][:])
                bq0 = wp.tile([P, 136], dt)
                nc.sync.dma_start(out=bq0[:], in_=wnames['l0_bq'][:].to_broadcast([P, 136]))
                bkv0 = wp.tile([P, 256], dt)
                nc.sync.dma_start(out=bkv0[:], in_=wnames['l0_bkv'][:].to_broadcast([P, 256]))
                bs0 = wp.tile([P, HD], dt)
                nc.sync.dma_start(out=bs0[:], in_=wnames['l0_bs'][:].to_broadcast([P, HD]))
                for t in range(NT_ALL):
                    xt = sp.tile([P, IN_CH], dt, tag="xt")
                    nc.sync.dma_start(out=xt[:], in_=x_in[t * P:(t + 1) * P, :])
                    xT_p = pp.tile([IN_CH, P], dt, tag="xT", space="PSUM")
                    nc.tensor.transpose(out=xT_p[:], in_=xt[:], identity=ident[:])
                    xT = sp.tile([IN_CH, P], dt, tag="xTs")
                    nc.vector.tensor_copy(out=xT[:], in_=xT_p[:])
                    pq = pp.tile([P, 136], dt, tag="pq", space="PSUM")
                    nc.tensor.matmul(pq[:], lhsT=xT[:], rhs=wq0[:], start=True, stop=True)
                    oq = sp.tile([P, 136], dt, tag="oq")
                    nc.vector.tensor_tensor(out=oq[:], in0=pq[:], in1=bq0[:], op=AL.add)
                    nc.sync.dma_start(out=tq0[t * P:(t + 1) * P, :], in_=oq[:])
                    pkv = pp.tile([P, 256], dt, tag="pkv", space="PSUM")
                    nc.tensor.matmul(pkv[:], lhsT=xT[:], rhs=wkv0[:], start=True, stop=True)
                    okv = sp.tile([P, 256], dt, tag="okv")
                    nc.vector.tensor_tensor(out=okv[:], in0=pkv[:], in1=bkv0[:], op=AL.add)
                    nc.sync.dma_start(out=tkv0[t * P:(t + 1) * P, :], in_=okv[:])
                    ps = pp.tile([P, HD], dt, tag="ps", space="PSUM")
                    nc.tensor.matmul(ps[:], lhsT=xT[:], rhs=ws0[:], start=True, stop=True)
                    os_ = sp.tile([P, HD], dt, tag="os")
                    nc.vector.tensor_tensor(out=os_[:], in0=ps[:], in1=bs0[:], op=AL.add)
                    nc.sync.dma_start(out=s0[t * P:(t + 1) * P, :], in_=os_[:])

            # ---------- edge pass (shared for both layers) ----------
            def edge_pass(tag, tq_tab, kv_tab, z_tab, src_in, dstq_in):
                with tc.tile_pool(name=f"eidx{tag}", bufs=2) as ep, \
                     tc.tile_pool(name=f"ebuf{tag}", bufs=3) as eb, \
                     tc.tile_pool(name=f"epp{tag}", bufs=2, space="PSUM") as pp:
                    for ch in range(T // CH):
                        srcT = ep.tile([P, CH], I32, tag="srcT")
                        nc.sync.dma_start(out=srcT[:], in_=src_in[ch])
                        dqT = ep.tile([P, CH], I32, tag="dqT")
                        nc.sync.dma_start(out=dqT[:], in_=dstq_in[ch])
                        dlT = ep.tile([P, CH], I32, tag="dlT")
                        nc.sync.dma_start(out=dlT[:], in_=dstl_in[ch])
                        eaT = ep.tile([P, CH], dt, tag="eaT")
                        nc.sync.dma_start(out=eaT[:], in_=eav_in[ch])
                        dlF = ep.tile([P, CH], dt, tag="dlF")
                        nc.vector.tensor_copy(out=dlF[:], in_=dlT[:])
                        for b in range(CH // B):
                            t0 = b * B
                            kvB = eb.tile([P, B, 256], dt, tag="kvB")
                            qB = eb.tile([P, B, 136], dt, tag="qB")
                            for j in range(B):
                                nc.gpsimd.indirect_dma_start(
                                    out=kvB[:, j, :], out_offset=None, in_=kv_tab[:],
                                    in_offset=IOO(ap=srcT[:, t0 + j:t0 + j + 1], axis=0))
                                nc.gpsimd.indirect_dma_start(
                                    out=qB[:, j, :], out_offset=None, in_=tq_tab[:],
                                    in_offset=IOO(ap=dqT[:, t0 + j:t0 + j + 1], axis=0))
                            # Sel build
                            tp = pp.tile([P, B, P], dt, tag="tp", space="PSUM")
                            for j in range(B):
                                nc.tensor.transpose(
                                    out=tp[:, j, :],
                                    in_=dlF[:, t0 + j:t0 + j + 1].to_broadcast([P, P]),
                                    identity=ident[:])
                            tsb = eb.tile([P, B, P], dt, tag="tsb")
                            nc.vector.tensor_copy(out=tsb[:], in_=tp[:])
                            sel = eb.tile([P, B, P], dt, tag="sel")
                            nc.vector.tensor_tensor(
                                out=sel[:],
                                in0=dlF[:, t0:t0 + B, None].to_broadcast([P, B, P]),
                                in1=tsb[:], op=AL.is_equal)
                            # logits / ex
                            prod = eb.tile([P, B, HD], dt, tag="prod")
                            nc.vector.tensor_tensor(
                                out=prod[:], in0=qB[:, :, 0:HD], in1=kvB[:, :, 0:HD],
                                op=AL.mult)
                            lg = eb.tile([P, B, H, 1], dt, tag="lg")
                            nc.vector.tensor_reduce(
                                out=lg[:],
                                in_=prod[:].rearrange("p b (h d) -> p b h d", h=H),
                                op=AL.add, axis=mybir.AxisListType.X)
                            eaq = eb.tile([P, B, H, 1], dt, tag="eaq")
                            nc.vector.tensor_tensor(
                                out=eaq[:], in0=qB[:, :, HD:HD + H, None],
                                in1=eaT[:, t0:t0 + B, None, None].to_broadcast([P, B, H, 1]),
                                op=AL.mult)
                            nc.vector.tensor_tensor(
                                out=lg[:], in0=lg[:], in1=eaq[:], op=AL.add)
                            ex = eb.tile([P, B, H, 1], dt, tag="ex")
                            nc.scalar.activation(ex[:], lg[:], ACT.Exp, scale=0.125)
                            # SelEx per head
                            selex = eb.tile([P, B, H, P], dt, tag="selex")
                            nc.vector.tensor_tensor(
                                out=selex[:],
                                in0=sel[:, :, None, :].to_broadcast([P, B, H, P]),
                                in1=ex[:].to_broadcast([P, B, H, P]),
                                op=AL.mult)
                            # onesea rhs [P, B, 2]
                            onea = eb.tile([P, B, 2], dt, tag="onea")
                            nc.vector.memset(onea[:, :, 0:1], 1.0)
                            nc.vector.tensor_copy(
                                out=onea[:, :, 1:2], in_=eaT[:, t0:t0 + B, None])
                            # combine matmuls
                            pv = pp.tile([P, B, HD], dt, tag="pv", space="PSUM")
                            psm = pp.tile([P, B, 4], dt, tag="psm", space="PSUM")
                            for j in range(B):
                                for h in range(H):
                                    nc.tensor.matmul(
                                        pv[:, j, h * D:(h + 1) * D],
                                        lhsT=selex[:, j, h, :],
                                        rhs=kvB[:, j, HD + h * D:HD + (h + 1) * D],
                                        start=True, stop=True)
                                    nc.tensor.matmul(
                                        psm[:, j, 2 * h:2 * h + 2],
                                        lhsT=selex[:, j, h, :],
                                        rhs=onea[:, j, :],
                                        start=True, stop=True)
                            sums = eb.tile([P, B, 136], dt, tag="sums")
                            nc.vector.tensor_copy(out=sums[:, :, 0:HD], in_=pv[:])
                            nc.vector.tensor_copy(out=sums[:, :, HD:HD + 4], in_=psm[:])
                            for j in range(B):
                                nc.gpsimd.indirect_dma_start(
                                    out=z_tab[:],
                                    out_offset=IOO(ap=dlT[:, t0 + j:t0 + j + 1], axis=0),
                                    in_=sums[:, j, :], in_offset=None)

            edge_pass("0", tq0, tkv0, z0,
                      [srcg_in[ch] for ch in range(T // CH)],
                      [dstg_in[ch] for ch in range(T // CH)])

            # ---------- node phase, layer 1 (own shard only) ----------
            with tc.tile_pool(name="np1", bufs=3) as sp, \
                 tc.tile_pool(name="np1w", bufs=1) as wp, \
                 tc.tile_pool(name="np1p", bufs=2, space="PSUM") as pp:
                wq1 = wp.tile([HD, 136], dt)
                nc.sync.dma_start(out=wq1[:], in_=wnames['l1_wq'][:])
                wkv1 = wp.tile([HD, 256], dt)
                nc.sync.dma_start(out=wkv1[:], in_=wnames['l1_wkv'][:])
                ws1 = wp.tile([HD, HD], dt)
                nc.sync.dma_start(out=ws1[:], in_=wnames['l1_ws'][:])
                bq1 = wp.tile([P, 136], dt)
                nc.sync.dma_start(out=bq1[:], in_=wnames['l1_bq'][:].to_broadcast([P, 136]))
                bkv1 = wp.tile([P, 256], dt)
                nc.sync.dma_start(out=bkv1[:], in_=wnames['l1_bkv'][:].to_broadcast([P, 256]))
                bs1 = wp.tile([P, HD], dt)
                nc.sync.dma_start(out=bs1[:], in_=wnames['l1_bs'][:].to_broadcast([P, HD]))
                we0b = wp.tile([P, HD], dt)
                nc.sync.dma_start(out=we0b[:], in_=wnames['l0_we'][:].to_broadcast([P, HD]))
                gidT = wp.tile([P, NLOC // P], I32)
                nc.sync.dma_start(out=gidT[:],
                                  in_=gids_in.ap().rearrange("a p o -> p (a o)"))
                for t in range(NT_LOC):
                    zr = sp.tile([P, 136], dt, tag="zr")
                    nc.sync.dma_start(out=zr[:], in_=z0[t * P:(t + 1) * P, :])
                    s0r = sp.tile([P, HD], dt, tag="s0r")
                    nc.gpsimd.indirect_dma_start(
                        out=s0r[:], out_offset=None, in_=s0[:],
                        in_offset=IOO(ap=gidT[:, t:t + 1], axis=0))
                    h1 = sp.tile([P, HD], dt, tag="h1")
                    _agg_relu(nc, sp, h1, zr, s0r, we0b)
                    hT_p = pp.tile([P, P], dt, tag="hT", space="PSUM")
                    nc.tensor.transpose(out=hT_p[:], in_=h1[:], identity=ident[:])
                    hT = sp.tile([P, P], dt, tag="hTs")
                    nc.vector.tensor_copy(out=hT[:], in_=hT_p[:])
                    pq = pp.tile([P, 136], dt, tag="pq1", space="PSUM")
                    nc.tensor.matmul(pq[:], lhsT=hT[:], rhs=wq1[:], start=True, stop=True)
                    oq = sp.tile([P, 136], dt, tag="oq1")
                    nc.vector.tensor_tensor(out=oq[:], in0=pq[:], in1=bq1[:], op=AL.add)
                    nc.sync.dma_start(out=tq1[t * P:(t + 1) * P, :], in_=oq[:])
                    pkv = pp.tile([P, 256], dt, tag="pkv1", space="PSUM")
                    nc.tensor.matmul(pkv[:], lhsT=hT[:], rhs=wkv1[:], start=True, stop=True)
                    okv = sp.tile([P, 256], dt, tag="okv1")
                    nc.vector.tensor_tensor(out=okv[:], in0=pkv[:], in1=bkv1[:], op=AL.add)
                    nc.sync.dma_start(out=tkv1_sh[t * P:(t + 1) * P, :], in_=okv[:])
                    ps = pp.tile([P, HD], dt, tag="ps1", space="PSUM")
                    nc.tensor.matmul(ps[:], lhsT=hT[:], rhs=ws1[:], start=True, stop=True)
                    os_ = sp.tile([P, HD], dt, tag="os1")
                    nc.vector.tensor_tensor(out=os_[:], in0=ps[:], in1=bs1[:], op=AL.add)
                    nc.sync.dma_start(out=s1[t * P:(t + 1) * P, :], in_=os_[:])

            nc.gpsimd.collective_compute(
                "AllGather", AL.bypass,
                replica_groups=[list(range(NC_CORES))],
                ins=[tkv1_sh.ap().opt()], outs=[tkv1_ag.ap().opt()])

            edge_pass("1", tq1, tkv1_ag, z1,
                      [srcag_in[ch] for ch in range(T // CH)],
                      [dstl_in[ch] for ch in range(T // CH)])

            # ---------- final: h2, pooling, MLP ----------
            with tc.tile_pool(name="fp", bufs=3) as sp, \
                 tc.tile_pool(name="fpw", bufs=1) as wp, \
                 tc.tile_pool(name="fpp", bufs=1, space="PSUM") as pp:
                we1b = wp.tile([P, HD], dt)
                nc.sync.dma_start(out=we1b[:], in_=wnames['l1_we'][:].to_broadcast([P, HD]))
                pg = pp.tile([N_GRAPHS, HD], dt, space="PSUM")
                for t in range(NT_LOC):
                    zr = sp.tile([P, 136], dt, tag="zr2")
                    nc.sync.dma_start(out=zr[:], in_=z1[t * P:(t + 1) * P, :])
                    s1r = sp.tile([P, HD], dt, tag="s1r")
                    nc.sync.dma_start(out=s1r[:], in_=s1[t * P:(t + 1) * P, :])
                    h2 = sp.tile([P, HD], dt, tag="h2")
                    _agg_relu(nc, sp, h2, zr, s1r, we1b)
                    oh = sp.tile([P, N_GRAPHS], dt, tag="oh")
                    nc.sync.dma_start(out=oh[:], in_=oneh_in[t])
                    nc.tensor.matmul(pg[:], lhsT=oh[:], rhs=h2[:],
                                     start=(t == 0), stop=(t == NT_LOC - 1))
                gs = wp.tile([N_GRAPHS, HD], dt)
                nc.vector.tensor_copy(out=gs[:], in_=pg[:])
                nc.sync.dma_start(out=gsum_b[:], in_=gs[:])
            nc.gpsimd.collective_compute(
                "AllReduce", AL.add,
                replica_groups=[list(range(NC_CORES))],
                ins=[gsum_b.ap().opt()], outs=[gsum_ag.ap().opt()])
            with tc.tile_pool(name="mlp", bufs=1) as sp, \
                 tc.tile_pool(name="mlpp", bufs=2, space="PSUM") as pp:
                g = sp.tile([N_GRAPHS, HD], dt)
                nc.sync.dma_start(out=g[:], in_=gsum_ag[:])
                cr = sp.tile([N_GRAPHS, 1], dt)
                nc.sync.dma_start(out=cr[:], in_=crec_in[:])
                nc.vector.tensor_scalar(out=g[:], in0=g[:], scalar1=cr[:],
                                        scalar2=None, op0=AL.mult)
                w1 = sp.tile([HD, 128], dt)
                nc.sync.dma_start(out=w1[:], in_=cW1_in[:])
                b1 = sp.tile([N_GRAPHS, 128], dt)
                nc.sync.dma_start(out=b1[:], in_=cb1_in[:].to_broadcast([N_GRAPHS, 128]))
                w2 = sp.tile([128, 32], dt)
                nc.sync.dma_start(out=w2[:], in_=cW2_in[:])
                b2 = sp.tile([N_GRAPHS, 32], dt)
                nc.sync.dma_start(out=b2[:], in_=cb2_in[:].to_broadcast([N_GRAPHS, 32]))
                gT_p = pp.tile([HD, N_GRAPHS], dt, tag="gT", space="PSUM")
                nc.tensor.transpose(out=gT_p[:], in_=g[:], identity=ident[:N_GRAPHS, :N_GRAPHS])
                gT = sp.tile([HD, N_GRAPHS], dt)
                nc.vector.tensor_copy(out=gT[:], in_=gT_p[:])
                p1 = pp.tile([N_GRAPHS, 128], dt, tag="p1", space="PSUM")
                nc.tensor.matmul(p1[:], lhsT=gT[:], rhs=w1[:], start=True, stop=True)
                y1 = sp.tile([N_GRAPHS, 128], dt)
                nc.vector.tensor_tensor(out=y1[:], in0=p1[:], in1=b1[:], op=AL.add)
                nc.scalar.activation(y1[:], y1[:], ACT.Relu)
                y1T_p = pp.tile([128, N_GRAPHS], dt, tag="y1T", space="PSUM")
                nc.tensor.transpose(out=y1T_p[:], in_=y1[:], identity=ident[:N_GRAPHS, :N_GRAPHS])
                y1T = sp.tile([128, N_GRAPHS], dt)
                nc.vector.tensor_copy(out=y1T[:], in_=y1T_p[:])
                p2 = pp.tile([N_GRAPHS, 32], dt, tag="p2", space="PSUM")
                nc.tensor.matmul(p2[:], lhsT=y1T[:], rhs=w2[:], start=True, stop=True)
                y2 = sp.tile([N_GRAPHS, 32], dt)
                nc.vector.tensor_tensor(out=y2[:], in0=p2[:], in1=b2[:], op=AL.add)
                nc.sync.dma_start(out=out[:], in_=y2[:])

    _legalize_waits(nc)
    return nc


def _agg_relu(nc, sp, h_out, zr, skip, we_b):
    """h = relu(Zv/(Zex+eps) + (Zexea/(Zex+eps))*We + skip).
    Z layout: [v(128) | ex0, exea0, ex1, exea1]."""
    dt = F32
    AL = mybir.AluOpType
    ACT = mybir.ActivationFunctionType
    zex = zr[:, HD:HD + 4].rearrange("p (h two) -> p h two", h=H)[:, :, 0:1]   # [P,H,1]
    zea = zr[:, HD:HD + 4].rearrange("p (h two) -> p h two", h=H)[:, :, 1:2]
    r = sp.tile([P, H, 1], dt, tag="rcp")
    nc.vector.tensor_scalar(out=r[:], in0=zex, scalar1=1e-16, scalar2=None, op0=AL.add)
    nc.vector.reciprocal(out=r[:], in_=r[:])
    rea = sp.tile([P, H, 1], dt, tag="rea")
    nc.vector.tensor_tensor(out=rea[:], in0=zea, in1=r[:], op=AL.mult)
    nc.vector.tensor_tensor(
        out=h_out[:].rearrange("p (h d) -> p h d", h=H),
        in0=zr[:, 0:HD].rearrange("p (h d) -> p h d", h=H),
        in1=r[:].to_broadcast([P, H, D]), op=AL.mult)
    wterm = sp.tile([P, HD], dt, tag="wterm")
    nc.vector.tensor_tensor(
        out=wterm[:].rearrange("p (h d) -> p h d", h=H),
        in0=we_b[:].rearrange("p (h d) -> p h d", h=H),
        in1=rea[:].to_broadcast([P, H, D]), op=AL.mult)
    nc.vector.tensor_tensor(out=h_out[:], in0=h_out[:], in1=wterm[:], op=AL.add)
    nc.vector.tensor_tensor(out=h_out[:], in0=h_out[:], in1=skip[:], op=AL.add)
    nc.scalar.activation(h_out[:], h_out[:], ACT.Relu)


_CACHE = {}


def kernel(**inputs):
    cores, T, NLOC, DUMMY, cnt_recip = _host_prep(inputs)
    w = _weights_host(inputs)

    key = (T, NLOC)
    if key not in _CACHE:
        _CACHE[key] = _build(T, NLOC, DUMMY)
    nc = _CACHE[key]

    xpad = np.zeros((NPAD, IN_CH), np.float32)
    xpad[:N_NODES] = np.asarray(inputs['x']).astype(np.float32)
    common = dict(
        x=xpad,
        cnt_recip=cnt_recip.reshape(N_GRAPHS, 1),
        ident=np.eye(P, dtype=np.float32),
        onescol=np.ones((P, 1), np.float32),
        cW1=w['cW1'], cb1=w['cb1'], cW2=w['cW2'], cb2=w['cb2'],
    )
    for li in ('l0', 'l1'):
        for nm in ('wq', 'bq', 'wkv', 'bkv', 'ws', 'bs', 'we'):
            common[f'{li}_{nm}'] = w[f'{li}_{nm}']

    in_maps = []
    for pc in cores:
        m = dict(common)
        m['srcg'] = _chunked_idx(pc['srcg'], T)
        m['srcag'] = _chunked_idx(pc['srcag'], T)
        m['dstg'] = _chunked_idx(pc['dstg'], T)
        m['dstl'] = _chunked_idx(pc['dstl'], T)
        m['eav'] = _chunked_idx(pc['eav'], T)
        m['own_gids'] = pc['own_gids'].reshape(NLOC // P, P, 1)
        m['onehot'] = pc['onehot']
        in_maps.append(m)

    res = run_bass_kernel_spmd(nc, in_maps, core_ids=list(range(NC_CORES)))
    out = np.asarray(res.results[0]['out'])[:, :N_CLASSES]
    return out.astype(np.float32)


if __name__ == "__main__":
    import reference  # only for standalone self-test; harness calls kernel()
    inp = {k: np.asarray(v) for k, v in reference.setup_inputs().items()}
    got = kernel(**inp)
    exp = np.asarray(reference.reference(**inp))
    err = np.abs(got - exp).max() / (np.abs(exp).max() + 1e-12)
    print("rel err:", err)
